# revision 9
# baseline (speedup 1.0000x reference)
"""Trainium2 Bass kernel for CounterfactualRepairAttention (v2).

Math (per batch sample b):
  valid/false/option segments from x_ids; gate = masked softmax over the
  false segment of (x @ Wa + ba); three QK attention score blocks; output is
  LayerNorm(MLP(concat(gate@x_f, gate@(rep_attn@x), gate@(sup_attn@x)))).

Structure (differences from v1 noted):
  * Attention runs on the [NF, NO] sub-block only; the per-type tail is two
    tall-skinny matvecs on E_t = exp(masked scores).
  * QK projections folded on host: S_t = x_f (Wq_t Wk_t^T) x_o^T with
    host-computed rank-1 bias terms.  Device: A_t = x_f M_t (fp8 DoubleRow),
    S_t = A_t x_o^T (fp8 DoubleRow).
  * E_rep = exp(s_rep + tanh(s_con)) computed directly (DVE add of the
    saved tanh block, then one Exp eviction with row-sum accumulation) —
    no separate exp(tanh) pass.
  * The anomaly gate is computed on the host (O(L*D) like the other host
    precomputes) and shipped in both row and partition layouts.
  * All DRAM tensors are pre-laid-out on host to match SBUF tiles, so every
    DMA is one contiguous run per partition (no small-descriptor floods).
  * Output is written [128, 6] partition-major (128 x 24B descriptors);
    host restores the [768] order.
  * PE instruction stream is kept dense (HAM stays un-throttled): dummy
    warm-up matmuls run during the DMA lead-in, per-block score evictions
    never sit between PE matmuls, and the MLP1 rank-1 stream fills the
    dependency shadows of the psw tails.
  * Data-parallel over the batch: one sample per NeuronCore, 8 cores.
"""

import math
import ml_dtypes
import numpy as np

BF = ml_dtypes.bfloat16
F8 = ml_dtypes.float8_e4m3

import concourse.bass as bass
import concourse.mybir as mybir
import concourse.tile as tile
from concourse import bacc
from concourse.bass_utils import run_bass_kernel_spmd

P = 128
D = 768
DC = D // P            # 6
TD = 3 * D             # 2304
TDC = TD // P          # 18
NEG = -9.0e15
MNEG = -1.0e15         # masking value injected into score PSUM
WS_M = 512.0           # host prescale of M = Wq @ Wk^T for fp8
WS_A = 0.125           # A eviction scale (fp8 range control)
SCL = 1.0 / (WS_M * WS_A * math.sqrt(D))   # score eviction scale
F32 = mybir.dt.float32
BF16 = mybir.dt.bfloat16
FP8 = mybir.dt.float8e4
AF = mybir.ActivationFunctionType
ALU = mybir.AluOpType
AX = mybir.AxisListType
DR = mybir.MatmulPerfMode.DoubleRow

NWARM = 6              # PE warm-up matmuls during the DMA lead-in


def _chunks(total, step):
    out = []
    o = 0
    while o < total:
        out.append((o, min(step, total - o)))
        o += step
    return out


def _build(NF, NO):
    """Per-core Bass program for padded segment sizes NF, NO (multiples of
    128, each <= 512).  Types are packed in order (con, rep, sup)."""
    assert NF <= 512 and NO <= 512
    NFC, NOC = NF // P, NO // P
    KS = DC // 2           # 3 DoubleRow k-steps over D
    NCONST = 3 * NFC + NFC + 3 * DC   # rbias | egpt | bf2t gammat betat
    nc = bacc.Bacc(None, target_bir_lowering=False)

    dxf8 = nc.dram_tensor("xf8", [P, DC, NF], FP8, kind="ExternalInput")
    dxo8 = nc.dram_tensor("xo8", [P, DC, NO], FP8, kind="ExternalInput")
    dxfb = nc.dram_tensor("xfb", [P, DC, NF], BF16, kind="ExternalInput")
    dxob = nc.dram_tensor("xob", [P, DC, NO], BF16, kind="ExternalInput")
    # [M_con | M_rep | M_sup], fp8, pre-scaled, partition-major
    dwm = nc.dram_tensor("wm", [P, 3, DC, D], FP8, kind="ExternalInput")
    # bf16 row pack: crow[3, NO] | g0row[NF] | bf1row[D]
    drow = nc.dram_tensor("row", [1, 3 * NO + NF + D], BF16,
                          kind="ExternalInput")
    # f32 per-partition consts: rbias[NFC,3] | egpt[NFC] | bf2t | gammat | betat
    dconst = nc.dram_tensor("cst", [P, NCONST], F32, kind="ExternalInput")
    dwf1 = nc.dram_tensor("wf1", [P, TDC, D], BF16, kind="ExternalInput")
    dwf2t = nc.dram_tensor("wf2t", [P, DC, D], BF16, kind="ExternalInput")
    dout = nc.dram_tensor("out", [P, DC], F32, kind="ExternalOutput")

    with tile.TileContext(nc) as tc:
        with (
            tc.tile_pool(name="const", bufs=1) as const,
            tc.tile_pool(name="xres", bufs=1) as xres,
            tc.tile_pool(name="at", bufs=2) as atp,
            tc.tile_pool(name="eres", bufs=1) as eres,
            tc.tile_pool(name="wstream", bufs=2) as wstream,
            tc.tile_pool(name="vecs", bufs=1) as vecs,
            tc.tile_pool(name="scratch", bufs=3) as scratch,
            tc.tile_pool(name="psbig", bufs=3, space="PSUM") as psbig,
            tc.tile_pool(name="psrow", bufs=2, space="PSUM") as psrow,
            tc.tile_pool(name="psvec", bufs=1, space="PSUM") as psvec,
            tc.tile_pool(name="psmlp", bufs=1, space="PSUM") as psmlp,
        ):
            # ---- warm-up fodder (no DMA deps): keeps the PE HAM busy ----
            wuw = const.tile([P, 2, 512], FP8)
            nc.vector.memset(wuw[:], 0.25)
            for w in range(NWARM):
                pswu = psbig.tile([P, 512], F32, tag="psbig", name=f"wu{w}")
                nc.tensor.matmul(pswu[:], wuw[:, :, 0:P], wuw[:, :, :],
                                 start=True, stop=True, perf_mode=DR)

            # ---- preload ACT tables during the DMA lead-in ----
            eps_sb = const.tile([1, 1], F32)
            nc.vector.memset(eps_sb[:], 1e-5)
            dum = vecs.tile([1, 3], F32)
            nc.scalar.activation(dum[:, 0:1], eps_sb[:], AF.Tanh)
            nc.scalar.activation(dum[:, 1:2], eps_sb[:], AF.Exp)
            nc.scalar.sqrt(dum[:, 2:3], eps_sb[:])

            # ---- first wave of loads (sync HWDGE queue, program order) ----
            sbxf8 = xres.tile([P, DC, NF], FP8)
            wm_t = [wstream.tile([P, DC, D], FP8, tag="wm", name=f"wm{t}")
                    for t in range(3)]
            for c2 in range(KS):
                nc.sync.dma_start(sbxf8[:, 2 * c2:2 * c2 + 2],
                                  dxf8[:, 2 * c2:2 * c2 + 2])
                nc.sync.dma_start(wm_t[0][:, 2 * c2:2 * c2 + 2],
                                  dwm[:, 0, 2 * c2:2 * c2 + 2])
            sbxo8 = xres.tile([P, DC, NO], FP8)
            nc.sync.dma_start(sbxo8[:], dxo8[:])
            nc.sync.dma_start(wm_t[1][:], dwm[:, 1])
            nc.sync.dma_start(wm_t[2][:], dwm[:, 2])
            sbxfb = xres.tile([P, DC, NF], BF16)
            nc.sync.dma_start(sbxfb[:], dxfb[:])
            sbxob = xres.tile([P, DC, NO], BF16)
            nc.sync.dma_start(sbxob[:], dxob[:])
            wf1_res = xres.tile([P, TDC, D], BF16)
            nc.sync.dma_start(wf1_res[:, 0:9], dwf1[:, 0:9])
            nc.sync.dma_start(wf1_res[:, 9:TDC], dwf1[:, 9:TDC])
            wf2t_res = xres.tile([P, DC, D], BF16)
            nc.sync.dma_start(wf2t_res[:], dwf2t[:])

            # small consts on the gpsimd software queue (early)
            row_sb = const.tile([1, 3 * NO + NF + D], BF16)
            nc.gpsimd.dma_start(row_sb[:], drow[:])
            cst_sb = const.tile([P, NCONST], F32)
            nc.gpsimd.dma_start(cst_sb[:], dconst[:])

            def crow(t):
                return row_sb[0:1, t * NO:(t + 1) * NO]

            g0row = row_sb[0:1, 3 * NO:3 * NO + NF]
            bf1row = row_sb[0:1, 3 * NO + NF:3 * NO + NF + D]

            def rbias(i, t):
                return cst_sb[:, (i * 3 + t):(i * 3 + t + 1)]

            egpt = cst_sb[:, 3 * NFC:4 * NFC]
            bf2t_sb = cst_sb[:, 4 * NFC:4 * NFC + DC]
            gammat_sb = cst_sb[:, 4 * NFC + DC:4 * NFC + 2 * DC]
            betat_sb = cst_sb[:, 4 * NFC + 2 * DC:4 * NFC + 3 * DC]

            ones_bf = const.tile([1, P], BF16)
            nc.vector.memset(ones_bf[:], 1.0)
            ones_col = const.tile([P, 1], F32)
            nc.vector.memset(ones_col[:], 1.0)

            # ---- shared tiles ----
            thall = eres.tile([P, NFC, NO], BF16)    # tanh(s_con) blocks
            E_rep = eres.tile([P, NFC, NO], BF16)
            E_sup = eres.tile([P, NFC, NO], BF16)
            r_rep = vecs.tile([P, NFC], F32)
            r_sup = vecs.tile([P, NFC], F32)
            fusedT = vecs.tile([P, TDC], F32)
            fusedT_bf = vecs.tile([P, TDC], BF16)
            nch = _chunks(D, 512)
            psh = {n0: psmlp.tile([1, 512], F32, tag=f"psh{n0}",
                                  name=f"psh{n0}")
                   for n0, _ in nch}

            def a_type(t):
                """A_t = x_f @ M_t, evicted to fp8 (x1/8), split S/V/G."""
                aT = atp.tile([P, DC, NF], FP8, tag="aT", name=f"aT{t}")
                for mc in range(DC):
                    psp = psbig.tile([P, 512], F32, tag="psbig")
                    for ks in range(KS):
                        nc.tensor.matmul(
                            psp[:, :NF],
                            wm_t[t][:, 2 * ks:2 * ks + 2, mc * P:(mc + 1) * P],
                            sbxf8[:, 2 * ks:2 * ks + 2, :],
                            start=(ks == 0), stop=(ks == KS - 1),
                            perf_mode=DR)
                    if mc % 2 == 0:
                        nc.scalar.mul(aT[:, mc, :], psp[:, :NF], WS_A)
                    else:
                        nc.vector.tensor_scalar(aT[:, mc, :], psp[:, :NF],
                                                WS_A, None, ALU.mult)
                return aT

            def score_mms(t, aT, i):
                """S psum for row block i of type t (colrow rank-1 + A x_o^T)."""
                pss = psbig.tile([P, 512], F32, tag="psbig")
                nc.tensor.matmul(pss[:, :NO], ones_bf[0:1, :],
                                 crow(t), start=True, stop=False)
                for ks in range(KS):
                    nc.tensor.matmul(
                        pss[:, :NO],
                        aT[:, 2 * ks:2 * ks + 2, i * P:(i + 1) * P],
                        sbxo8[:, 2 * ks:2 * ks + 2, :],
                        start=False, stop=(ks == KS - 1),
                        perf_mode=DR)
                return pss

            def fused_section(sec, g_bc, xTb, NN):
                """fusedT[:, 6*sec:6*sec+6] = x^T @ g; split DVE / GpSimd."""
                for c in range(DC):
                    eng = nc.vector
                    scr = scratch.tile([P, 512], BF16, tag=f"stt{c % 2}")
                    eng.scalar_tensor_tensor(
                        scr[:, :NN], xTb[:, c, :], 1.0, g_bc[:, :],
                        ALU.mult, ALU.mult,
                        accum_out=fusedT[:, sec * DC + c:sec * DC + c + 1])
                nc.gpsimd.tensor_copy(fusedT_bf[:, sec * DC:(sec + 1) * DC],
                                      fusedT[:, sec * DC:(sec + 1) * DC])

            def mlp1(c0, c1):
                for c in range(c0, c1):
                    for n0, nsz in nch:
                        nc.tensor.matmul(psh[n0][:, :nsz],
                                         fusedT_bf[:, c:c + 1],
                                         wf1_res[:, c, n0:n0 + nsz],
                                         start=(c == 0), stop=False)

            # gate broadcast (host-computed), needed for section 0
            g0_bc = vecs.tile([P, NF], BF16)
            nc.gpsimd.partition_broadcast(g0_bc[:], g0row)

            # ---- type 0 (con): A then scores; tanh blocks saved ----
            aT0 = a_type(0)
            for i in range(NFC):
                pss = score_mms(0, aT0, i)
                nc.scalar.activation(thall[:, i, :], pss[:, :NO], AF.Tanh,
                                     bias=rbias(i, 0), scale=SCL)

            # ---- type 1 (rep): E_rep = exp(s*SCL + th + rbias) ----
            aT1 = a_type(1)
            fused_section(0, g0_bc, sbxfb, NF)
            for i in range(NFC):
                pss = score_mms(1, aT1, i)
                tmp = scratch.tile([P, 512], BF16, tag="tmp")
                nc.vector.scalar_tensor_tensor(
                    tmp[:, :NO], pss[:, :NO], SCL, thall[:, i, :],
                    ALU.mult, ALU.add)
                nc.scalar.activation(E_rep[:, i, :], tmp[:, :NO], AF.Exp,
                                     bias=rbias(i, 1), scale=1.0,
                                     accum_out=r_rep[:, i:i + 1])

            # ---- type 2 (sup) A; rep tail deps resolve meanwhile ----
            aT2 = a_type(2)
            rcp1 = vecs.tile([P, NFC], F32)
            nc.vector.reciprocal(rcp1[:], r_rep[:])
            g_rep = vecs.tile([P, NFC], BF16)
            nc.vector.tensor_mul(g_rep[:], egpt, rcp1[:])
            psw1 = psrow.tile([1, 512], F32, tag="psrow", name="psw1")
            for i in range(NFC):
                nc.tensor.matmul(psw1[:, :NO], g_rep[:, i:i + 1],
                                 E_rep[:, i, :],
                                 start=(i == 0), stop=(i == NFC - 1))
            wv_rep = vecs.tile([1, NO], BF16)
            nc.vector.tensor_copy(wv_rep[:], psw1[:, :NO])
            wv_rep_bc = vecs.tile([P, NO], BF16)
            nc.gpsimd.partition_broadcast(wv_rep_bc[:], wv_rep[:])

            # ---- sup scores; then psw2; mlp1 fills the shadows ----
            for i in range(NFC):
                pss = score_mms(2, aT2, i)
                nc.scalar.activation(E_sup[:, i, :], pss[:, :NO], AF.Exp,
                                     bias=rbias(i, 2), scale=SCL,
                                     accum_out=r_sup[:, i:i + 1])
            fused_section(1, wv_rep_bc, sbxob, NO)
            mlp1(0, DC)
            rcp2 = vecs.tile([P, NFC], F32)
            nc.vector.reciprocal(rcp2[:], r_sup[:])
            g_sup = vecs.tile([P, NFC], BF16)
            nc.vector.tensor_mul(g_sup[:], egpt, rcp2[:])
            psw2 = psrow.tile([1, 512], F32, tag="psrow", name="psw2")
            for i in range(NFC):
                nc.tensor.matmul(psw2[:, :NO], g_sup[:, i:i + 1],
                                 E_sup[:, i, :],
                                 start=(i == 0), stop=(i == NFC - 1))
            mlp1(DC, 2 * DC)
            wv_sup = vecs.tile([1, NO], BF16)
            nc.vector.tensor_copy(wv_sup[:], psw2[:, :NO])
            wv_sup_bc = vecs.tile([P, NO], BF16)
            nc.gpsimd.partition_broadcast(wv_sup_bc[:], wv_sup[:])
            fused_section(2, wv_sup_bc, sbxob, NO)
            mlp1(2 * DC, TDC)
            # bf1 into the same PSUM accumulation (closes the group)
            for n0, nsz in nch:
                nc.tensor.matmul(psh[n0][:, :nsz], ones_bf[0:1, 0:1],
                                 bf1row[0:1, n0:n0 + nsz],
                                 start=False, stop=True)

            # ---- h = relu(psh) row, broadcast ----
            h_row = vecs.tile([1, D], BF16)
            for n0, nsz in nch:
                nc.vector.tensor_scalar(h_row[0:1, n0:n0 + nsz],
                                        psh[n0][:, :nsz], 0.0, None, ALU.max)
            h_bc = vecs.tile([P, D], BF16)
            nc.gpsimd.partition_broadcast(h_bc[:], h_row[:])

            # ---- MLP2: oT[p, j] = sum_c h[c] Wf2[c, j*128+p]; V/G split ----
            oT = vecs.tile([P, DC], F32)
            for j in range(DC):
                eng = nc.vector
                scr = scratch.tile([P, D], BF16, tag=f"stt2{j % 2}")
                eng.scalar_tensor_tensor(
                    scr[:], wf2t_res[:, j, :], 1.0, h_bc[:, :],
                    ALU.mult, ALU.mult, accum_out=oT[:, j:j + 1])
            nc.vector.tensor_add(oT[:], oT[:], bf2t_sb)

            # ---- LayerNorm on [128, 6] partition layout ----
            sqT = vecs.tile([P, DC], F32)
            nc.vector.tensor_mul(sqT[:], oT[:], oT[:])
            ps6 = psvec.tile([1, 2 * DC], F32, tag="psvec", name="ps6")
            nc.tensor.matmul(ps6[:, 0:DC], ones_col[:, 0:1], oT[:],
                             start=True, stop=True)
            nc.tensor.matmul(ps6[:, DC:2 * DC], ones_col[:, 0:1], sqT[:],
                             start=True, stop=True)
            ssums = vecs.tile([1, 2], F32)
            nc.vector.reduce_sum(ssums[:, 0:1], ps6[:, 0:DC], axis=AX.X)
            nc.vector.reduce_sum(ssums[:, 1:2], ps6[:, DC:2 * DC], axis=AX.X)
            murs = vecs.tile([1, 2], F32)
            mu = murs[:, 0:1]
            nc.vector.tensor_scalar(mu, ssums[:, 0:1], 1.0 / D, None,
                                    ALU.mult)
            esq = vecs.tile([1, 1], F32)
            nc.vector.tensor_scalar(esq[:], ssums[:, 1:2], 1.0 / D, None,
                                    ALU.mult)
            mu2 = vecs.tile([1, 1], F32)
            nc.vector.tensor_mul(mu2[:], mu, mu)
            var = vecs.tile([1, 1], F32)
            nc.vector.tensor_scalar(var[:], esq[:], mu2[0:1, 0:1], None,
                                    ALU.subtract)
            sd = vecs.tile([1, 1], F32)
            nc.scalar.activation(sd[:], var[:], AF.Sqrt, bias=eps_sb[0:1, 0:1],
                                 scale=1.0)
            nc.vector.reciprocal(murs[:, 1:2], sd[:])
            murs_bc = vecs.tile([P, 2], F32)
            nc.gpsimd.partition_broadcast(murs_bc[:], murs[:])
            onrm = vecs.tile([P, DC], F32)
            nc.vector.tensor_scalar(onrm[:], oT[:], murs_bc[:, 0:1],
                                    murs_bc[:, 1:2], ALU.subtract, ALU.mult)
            nc.vector.tensor_mul(onrm[:], onrm[:], gammat_sb)
            nc.vector.tensor_add(onrm[:], onrm[:], betat_sb)
            nc.sync.dma_start(dout[:, :], onrm[:])

    nc.finalize()
    return nc


_BUILD_CACHE = {}
_LAST_IN_MAPS = None  # captured for external profiling harnesses


def _get_program(NF, NO):
    key = (NF, NO)
    if key not in _BUILD_CACHE:
        _BUILD_CACHE[key] = _build(NF, NO)
    return _BUILD_CACHE[key]


def _np_softmax(x, axis):
    m = np.max(x, axis=axis, keepdims=True)
    e = np.exp(x - m)
    return e / e.sum(axis=axis, keepdims=True)


def _reference_numpy_sample(x, ids, pad_idx, W):
    """Full numpy replica of the reference for one sample (fallback for
    degenerate segment cases)."""
    L, d = x.shape
    valid = ids != pad_idx
    sep = int(np.clip(valid.sum() // 2, 1, max(1, L - 2)))
    pos = np.arange(L)
    fm = (pos < sep) & valid
    om = (pos > sep) & valid
    a = (x @ W["Wa"] + W["ba"])[:, 0]
    a = np.where(fm, a, NEG)
    gate = _np_softmax(a, 0) * fm
    gate = gate / max(gate.sum(), 1e-8)
    scale = 1.0 / math.sqrt(d)
    qs, ks = x @ W["Wqs"] + W["bqs"], x @ W["Wks"] + W["bks"]
    qc, kc = x @ W["Wqc"] + W["bqc"], x @ W["Wkc"] + W["bkc"]
    qr, kr = x @ W["Wqr"] + W["bqr"], x @ W["Wkr"] + W["bkr"]
    sup_s = qs @ ks.T * scale
    con_s = qc @ kc.T * scale
    rep_s = qr @ kr.T * scale
    pm = fm[:, None] & om[None, :]
    sup_attn = _np_softmax(np.where(pm, sup_s, NEG), 1)
    rep_attn = _np_softmax(np.where(pm, rep_s + np.tanh(con_s), NEG), 1)
    rep_vec = rep_attn @ x
    sup_vec = sup_attn @ x
    fused = np.concatenate([gate @ x, gate @ rep_vec, gate @ sup_vec])
    fused = np.maximum(fused @ W["Wf1"] + W["bf1"], 0.0) @ W["Wf2"] + W["bf2"]
    mu = fused.mean()
    var = ((fused - mu) ** 2).mean()
    return (fused - mu) / np.sqrt(var + 1e-5) * W["gamma"] + W["beta"]


def _pT(vec, nchunks):
    """[nchunks*128] -> [128, nchunks] partition layout."""
    return np.ascontiguousarray(vec.reshape(nchunks, P).T)


def kernel(**inputs):
    x = np.ascontiguousarray(np.asarray(inputs["x"], dtype=np.float32))
    x_ids = np.asarray(inputs["x_ids"])
    pad_idx = int(np.asarray(inputs["pad_idx"]))
    B, L, d = x.shape
    assert d == D

    W = {k: np.asarray(inputs[k], dtype=np.float32) for k in (
        "Wa", "ba", "Wqs", "bqs", "Wks", "bks", "Wqc", "bqc", "Wkc", "bkc",
        "Wqr", "bqr", "Wkr", "bkr", "Wf1", "bf1", "Wf2", "bf2", "gamma",
        "beta")}

    # folded score matrices, packed type order (con, rep, sup), plus the
    # rank-1 bias vectors: a = x_f @ (Wq bk), b = x_o @ (Wk bq), c = bq.bk
    Ms, cs, uvec, vvec = [], [], [], []
    for qn, kn in (("Wqc", "Wkc"), ("Wqr", "Wkr"), ("Wqs", "Wks")):
        bqn, bkn = "b" + qn[1:], "b" + kn[1:]
        Wq64 = W[qn].astype(np.float64)
        Wk64 = W[kn].astype(np.float64)
        Ms.append((Wq64 @ Wk64.T).astype(np.float32))
        cs.append(float(W[bqn].astype(np.float64) @ W[bkn].astype(np.float64)))
        uvec.append((Wq64 @ W[bkn].astype(np.float64)).astype(np.float32))
        vvec.append((Wk64 @ W[bqn].astype(np.float64)).astype(np.float32))

    pos = np.arange(L)
    per_sample = []
    fallback = {}
    max_nf, max_no = 0, 0
    for b in range(B):
        valid = x_ids[b] != pad_idx
        sep = int(np.clip(int(valid.sum()) // 2, 1, max(1, L - 2)))
        fi = np.nonzero((pos < sep) & valid)[0]
        oi = np.nonzero((pos > sep) & valid)[0]
        if len(oi) == 0 or len(fi) == 0 or len(fi) > 512 or len(oi) > 512:
            fallback[b] = _reference_numpy_sample(
                x[b].astype(np.float64), x_ids[b], pad_idx,
                {k: v.astype(np.float64) for k, v in W.items()})
            per_sample.append(None)
            continue
        per_sample.append((fi, oi))
        max_nf = max(max_nf, len(fi))
        max_no = max(max_no, len(oi))

    out = np.zeros((B, D), dtype=np.float32)
    live = [b for b in range(B) if per_sample[b] is not None]
    if live:
        NF = max(P, ((max_nf + P - 1) // P) * P)
        NO = max(P, ((max_no + P - 1) // P) * P)
        NFC, NOC = NF // P, NO // P
        nc = _get_program(NF, NO)

        def part_major(aT_cn):
            # [D, N] -> [P, DC, N] partition-major
            Dd, N = aT_cn.shape
            return np.ascontiguousarray(
                aT_cn.reshape(DC, P, N).transpose(1, 0, 2))

        # fp8 weight pack [P, 3, DC, D]
        wm = np.stack([part_major(np.clip(M * WS_M, -240, 240)) for M in Ms],
                      axis=1).astype(F8)
        # wf1 [P, TDC, D]: Wf1[c*128+p, n]
        wf1p = np.ascontiguousarray(
            W["Wf1"].reshape(TDC, P, D).transpose(1, 0, 2)).astype(BF)
        # wf2t [P, DC, D]: Wf2[c, j*128+p]
        wf2p = np.ascontiguousarray(
            W["Wf2"].T.reshape(DC, P, D).transpose(1, 0, 2)).astype(BF)
        shared = {"wm": wm, "wf1": wf1p, "wf2t": wf2p}
        cst_tail = np.concatenate(
            [_pT(W["bf2"], DC), _pT(W["gamma"], DC), _pT(W["beta"], DC)],
            axis=1)

        in_maps_all = []
        for b in live:
            fi, oi = per_sample[b]
            xf = np.zeros((NF, D), np.float32)
            xf[:len(fi)] = x[b, fi]
            xo = np.zeros((NO, D), np.float32)
            xo[:len(oi)] = x[b, oi]
            omask = np.zeros(NO, np.float32)
            omask[:len(oi)] = 1.0
            xfT = xf.T
            xoT = xo.T
            # host gate (fp64 softmax over the false rows)
            a = xf[:len(fi)].astype(np.float64) @ W["Wa"][:, 0].astype(
                np.float64) + float(W["ba"][0])
            e = np.exp(a - a.max())
            gate = np.zeros(NF, np.float64)
            gate[:len(fi)] = e / max(e.sum(), 1e-8)
            # score per-column bias rows: (b_t[n] + c_t)*scale/SCL + mask NEG
            sc = 1.0 / math.sqrt(D)
            crow = np.zeros((3, NO), np.float32)
            for t in range(3):
                bt = xo @ vvec[t] + cs[t]
                crow[t] = bt * sc / SCL
                if t >= 1:
                    crow[t] += (1.0 - omask) * MNEG
            # per-row bias columns a_t[l]*scale, layout [P, NFC*3]
            rb = np.zeros((NF, 3), np.float32)
            for t in range(3):
                rb[:, t] = (xf @ uvec[t]) * sc
            rbias = np.ascontiguousarray(
                rb.reshape(NFC, P, 3).transpose(1, 0, 2)).reshape(P, NFC * 3)
            cst = np.concatenate(
                [rbias, _pT(gate.astype(np.float32), NFC), cst_tail], axis=1)
            rowpk = np.concatenate(
                [crow.reshape(-1), gate, np.zeros(D)]).astype(BF)
            rowpk[3 * NO + NF:] = W["bf1"].astype(BF)
            in_maps_all.append(dict(
                shared,
                xf8=part_major(np.clip(xfT, -240, 240)).astype(F8),
                xo8=part_major(np.clip(xoT, -240, 240)).astype(F8),
                xfb=part_major(xfT).astype(BF),
                xob=part_major(xoT).astype(BF),
                row=rowpk.reshape(1, -1),
                cst=np.ascontiguousarray(cst),
            ))
        global _LAST_IN_MAPS
        _LAST_IN_MAPS = in_maps_all
        for r0 in range(0, len(live), 8):
            batch = in_maps_all[r0:r0 + 8]
            res = run_bass_kernel_spmd(nc, batch,
                                       core_ids=list(range(len(batch))))
            for k, b in enumerate(live[r0:r0 + 8]):
                out[b] = res.results[k]["out"].T.reshape(D)
    for b, v in fallback.items():
        out[b] = v.astype(np.float32)
    return out
